# Initial kernel scaffold
#
"""NLIF recurrent network kernel for 8 TRN2 NeuronCores.

Data-parallel over batch (8 rows/core, weights replicated, no collectives).
Per step: I/10 = S@(Ws/100) + gdv@(Wf/10) + (x@W_in)/10 via col-tiled
float32r matmuls (activation-stationary, weights streaming), PE transposes
to neuron-major, short DVE update chain.

State scaling: S = 10*s so s' = 0.9s + 0.1*gdv becomes S' = 0.9S + gdv.
s_fast' = gdv exactly (from s_fast + (gdv - s_fast)).
"""

import numpy as np

import concourse.bass as bass
import concourse.mybir as mybir
import concourse.tile as tile
from concourse import bacc
from concourse import bass_utils

# problem constants (hardcoded per spec)
N = 1024
T = 128
B = 64
NCORES = 8
BL = B // NCORES          # batch rows per core = 8
KC = N // 128             # contraction chunks = 8
NT = 4                    # col tiles per K-chunk (tile_position col groups)
CW = 1024 // NT           # cols per col tile = 256

F32 = mybir.dt.float32
F32R = mybir.dt.float32r
USE_F32R = True           # matmul moving/stationary dtype (4x faster than f32)


def _mm_ap(ap):
    return ap.bitcast(F32R) if USE_F32R else ap


def build(nsteps=T):
    nc = bacc.Bacc("TRN2", target_bir_lowering=False, debug=False,
                   num_devices=NCORES)

    TW = nsteps * BL * KC  # free width of time-major buffers (nsteps*64)

    # DRAM I/O
    d_ws = nc.dram_tensor("wsyn", [128, KC * N], F32, kind="ExternalInput")
    d_wf = nc.dram_tensor("wfast", [128, KC * N], F32, kind="ExternalInput")
    d_xp = nc.dram_tensor("xp", [128, TW], F32, kind="ExternalInput")
    d_ot = nc.dram_tensor("o10t", [128, 2 * KC], F32, kind="ExternalInput")
    d_ey = nc.dram_tensor("eye8", [8, 8], F32, kind="ExternalInput")
    d_spk = nc.dram_tensor("spk", [128, TW], F32, kind="ExternalOutput")
    d_ro = nc.dram_tensor("ro", [2, nsteps * BL], F32, kind="ExternalOutput")

    # persistent SBUF
    w_s = nc.alloc_sbuf_tensor("w_s", [128, KC * N], F32)
    w_f = nc.alloc_sbuf_tensor("w_f", [128, KC * N], F32)
    xp_s = nc.alloc_sbuf_tensor("xp_s", [128, TW], F32)
    s_hist = nc.alloc_sbuf_tensor("s_hist", [128, (nsteps + 1) * 64], F32)
    spk_s = nc.alloc_sbuf_tensor("spk_s", [128, TW], F32)
    ot_s = nc.alloc_sbuf_tensor("ot_s", [128, 2 * KC], F32)
    ey_s = nc.alloc_sbuf_tensor("ey_s", [8, 8], F32)
    ro_s = nc.alloc_sbuf_tensor("ro_s", [2, nsteps * BL], F32)

    def pair(name, shape):
        return [nc.alloc_sbuf_tensor(f"{name}{i}", shape, F32) for i in range(2)]

    gdvb = pair("gdv", [128, 64])
    vb = pair("v", [128, 64])
    v01b = pair("v01", [128, 64])
    cb = pair("c", [128, 64])
    t9b = pair("t9", [128, 64])
    foldb = pair("fold", [128, NT * CW // 4])   # [128, 256]
    dvb = pair("dv", [128, 64])
    vnb = pair("vn", [128, 64])
    gb = pair("g", [128, 64])
    t2b = pair("t2", [128, 64])

    psA = [nc.alloc_psum_tensor(f"psA{i}", [128, CW], F32) for i in range(2)]
    psB = [nc.alloc_psum_tensor(f"psB{i}", [128, 64], F32) for i in range(2)]
    psR = nc.alloc_psum_tensor("psR", [2, 512], F32)

    AT = mybir.AluOpType

    with tile.TileContext(nc) as tc:
        # input DMAs
        nc.sync.dma_start(w_s.ap(), d_ws.ap())
        nc.sync.dma_start(w_f.ap(), d_wf.ap())
        nc.sync.dma_start(xp_s.ap(), d_xp.ap())
        nc.sync.dma_start(ot_s.ap(), d_ot.ap())
        nc.sync.dma_start(ey_s.ap(), d_ey.ap())

        # zero init
        for z in (gdvb[0], gdvb[1], vb[0], vb[1]):
            nc.vector.memset(z.ap(), 0.0)
        nc.vector.memset(s_hist.ap()[:, 0:64], 0.0)
        nc.vector.memset(psA[0].ap(), 0.0)
        nc.vector.memset(psA[1].ap(), 0.0)

        for t in range(nsteps):
            cur, prv = t % 2, (t + 1) % 2
            stat_s = s_hist.ap()[:, t * 64:(t + 1) * 64]
            stat_f = gdvb[prv].ap()
            pA = psA[cur].ap()
            pB = psB[cur].ap()
            xp_t = xp_s.ap()[:, t * 64:(t + 1) * 64]

            # off-path ops (feed the on-path chain)
            nc.scalar.mul(t9b[cur].ap(), stat_s, 0.9)
            nc.scalar.mul(v01b[cur].ap(), vb[prv].ap(), 0.1)
            nc.gpsimd.tensor_tensor(cb[cur].ap(), xp_t, v01b[cur].ap(),
                                    AT.subtract)

            # matmuls: accumulate I/10 into pA (folded col-tiled layout)
            for k in range(KC):
                for wi, (wt, st) in enumerate(((w_s, stat_s), (w_f, stat_f))):
                    lhsT = _mm_ap(st[:, k * 8:(k + 1) * 8])
                    for ct in range(NT):
                        rhs = _mm_ap(wt.ap()[:, k * N + ct * CW: k * N + (ct + 1) * CW])
                        nc.tensor.matmul(
                            pA[32 * ct:32 * ct + 8, :],
                            lhsT, rhs,
                            start=(k == 0 and wi == 0),
                            stop=(k == KC - 1 and wi == 1),
                            tile_position=(0, 32 * ct),
                        )

            # PSUM -> SBUF (folded), then 8 PE transposes -> pB [128, 64]
            fold = foldb[cur].ap()
            nc.scalar.mul(fold, pA, 1.0)
            for k in range(KC):
                src = fold[32 * (k // 2):32 * (k // 2) + 8,
                           (k % 2) * 128:(k % 2) * 128 + 128]
                nc.tensor.transpose(pB[:, k * 8:(k + 1) * 8], src, ey_s.ap())

            # on-path update chain (DVE)
            dv, vn, g = dvb[cur].ap(), vnb[cur].ap(), gb[cur].ap()
            nc.vector.tensor_tensor(dv, pB, cb[cur].ap(), AT.add)
            nc.vector.tensor_tensor(vn, vb[prv].ap(), dv, AT.add)
            nc.vector.tensor_scalar(g, vn, -1.0, 1.0, AT.max, AT.min)
            nc.vector.tensor_tensor(gdvb[cur].ap(), g, dv, AT.mult)
            nc.vector.tensor_tensor(s_hist.ap()[:, (t + 1) * 64:(t + 2) * 64],
                                    t9b[cur].ap(), gdvb[cur].ap(), AT.add)

            # off-path: spikes and v update
            spk_t = spk_s.ap()[:, t * 64:(t + 1) * 64]
            nc.gpsimd.tensor_scalar(spk_t, vn, 0.0, 1.0, AT.abs_max, AT.is_ge)
            nc.gpsimd.tensor_scalar(t2b[cur].ap(), spk_t, -1.0, 2.0,
                                    AT.mult, AT.add)
            nc.gpsimd.tensor_tensor(vb[cur].ap(), t2b[cur].ap(), vn, AT.mult)

        # readout: ro^T[o, t*8+b] = sum_k O10T[k].T @ S_hist ; two 512-col halves
        nh = max(1, (nsteps * BL) // 512)
        hw_ = min(512, nsteps * BL)
        for h in range(nh):
            for k in range(KC):
                lhsT = _mm_ap(ot_s.ap()[:, k * 2:(k + 1) * 2])
                # rhs: S_hist slots (t+1), strided AP [128][t: hw_/8, stride 64][b: 8]
                base = 64 + h * (hw_ * 8)
                rhs = _mm_ap(s_hist.ap()[:, base:base + hw_ * 8]
                             .reshape([128, hw_ // 8, 8, 8])[:, :, k, :]
                             .reshape([128, hw_]))
                nc.tensor.matmul(psR.ap()[:, 0:hw_], lhsT, rhs,
                                 start=(k == 0), stop=(k == KC - 1))
            nc.vector.tensor_copy(ro_s.ap()[:, h * hw_:(h + 1) * hw_],
                                  psR.ap()[:, 0:hw_])

        # outputs
        nc.sync.dma_start(d_spk.ap(), spk_s.ap())
        nc.sync.dma_start(d_ro.ap(), ro_s.ap())

    nc.compile()
    return nc


def prep_inputs(x_in, W_syn, W_fast, W_in, O, nsteps=T):
    x_in = np.asarray(x_in, dtype=np.float32)
    W_syn = np.asarray(W_syn, dtype=np.float32)
    W_fast = np.asarray(W_fast, dtype=np.float32)
    W_in = np.asarray(W_in, dtype=np.float32)
    O = np.asarray(O, dtype=np.float32)

    mask = 1.0 - np.eye(N, dtype=np.float32)
    ws = ((W_syn * mask) / 100.0).astype(np.float32)
    wf = ((W_fast * mask) / 10.0).astype(np.float32)
    # [p, k*1024+n] = W[k*128+p, n]
    ws_l = ws.reshape(KC, 128, N).transpose(1, 0, 2).reshape(128, KC * N).copy()
    wf_l = wf.reshape(KC, 128, N).transpose(1, 0, 2).reshape(128, KC * N).copy()

    XP = (x_in[:nsteps].reshape(nsteps * B, 2) @ W_in).reshape(nsteps, B, N) / 10.0
    XP = XP.astype(np.float32)

    ot = (O / 10.0).astype(np.float32)
    ot_l = ot.reshape(2, KC, 128).transpose(2, 1, 0).reshape(128, 2 * KC).copy()
    eye8 = np.eye(8, dtype=np.float32)

    in_maps = []
    for c in range(NCORES):
        xc = XP[:, c * BL:(c + 1) * BL, :]              # [t, b, n]
        xc = xc.reshape(nsteps, BL, KC, 128)            # [t, b, k, p]
        xc = xc.transpose(3, 0, 2, 1).reshape(128, nsteps * 64).copy()
        in_maps.append({
            "wsyn": ws_l, "wfast": wf_l, "xp": xc,
            "o10t": ot_l, "eye8": eye8,
        })
    return in_maps


def assemble(results, nsteps=T):
    spikes = np.empty((nsteps, B, N), dtype=np.float32)
    readout = np.empty((nsteps, B, 2), dtype=np.float32)
    for c in range(NCORES):
        spk = results[c]["spk"].reshape(128, nsteps, KC, BL)   # [p, t, k, b]
        spikes[:, c * BL:(c + 1) * BL, :] = (
            spk.transpose(1, 3, 2, 0).reshape(nsteps, BL, N))
        ro = results[c]["ro"].reshape(2, nsteps, BL)           # [o, t, b]
        readout[:, c * BL:(c + 1) * BL, :] = ro.transpose(1, 2, 0)
    return spikes, readout


_NC_CACHE = {}


def kernel(x_in, W_syn, W_fast, W_in, O):
    nsteps = x_in.shape[0]
    if nsteps not in _NC_CACHE:
        _NC_CACHE[nsteps] = build(nsteps)
    nc = _NC_CACHE[nsteps]
    in_maps = prep_inputs(x_in, W_syn, W_fast, W_in, O, nsteps)
    res = bass_utils.run_bass_kernel_spmd(
        nc, in_maps, core_ids=list(range(NCORES)))
    return assemble(res.results, nsteps)


# revision 10
# speedup vs baseline: 1.0427x; 1.0427x over previous
"""NLIF recurrent network kernel for 8 TRN2 NeuronCores.

Data-parallel over batch (8 rows/core, weights replicated, no collectives).
Per step: I/10 = S@(Ws/100) + gdv@(Wf/10) + (x@W_in)/10 via col-tiled
float32r matmuls (activation-stationary, weights streaming), PE transposes
to neuron-major, short DVE update chain.

State scaling: S = 10*s so s' = 0.9s + 0.1*gdv becomes S' = 0.9S + gdv.
s_fast' = gdv exactly (from s_fast + (gdv - s_fast) in the reference).
"""

import os
import numpy as np

import concourse.bass as bass
import concourse.mybir as mybir
import concourse.tile as tile
from concourse import bacc
from concourse import bass_utils

# problem constants (hardcoded per spec)
N = 1024
T = 128
B = 64
NCORES = 8
BL = B // NCORES          # batch rows per core = 8
KC = N // 128             # contraction chunks = 8
F32 = mybir.dt.float32
F32R = mybir.dt.float32r
USE_F32R = os.environ.get("NLIF_F32R", "1") == "1"  # matmul dtype
NT = 1 if USE_F32R else 4  # col tiles (f32r requires dst partition base 0)
CW = N // NT              # cols per col tile
PER_T = KC // NT          # K-chunks whose transpose source shares a col tile


MMD = F32R if USE_F32R else F32


def round_f32r(x):
    """Round fp32 array to e8m11 (FP32R) with round-to-nearest-even."""
    if not USE_F32R:
        return np.ascontiguousarray(x, np.float32)
    u = np.ascontiguousarray(x, np.float32).view(np.uint32)
    low = u & 0xFFF
    hi = u >> 12
    carry = (low > 0x800) | ((low == 0x800) & ((hi & 1) == 1))
    return ((hi + carry.astype(np.uint32)) << 12).view(np.float32)


def build(nsteps=T):
    nc = bacc.Bacc("TRN2", target_bir_lowering=False, debug=False,
                   num_devices=NCORES)

    TW = nsteps * 64  # free width of time-major buffers

    # DRAM I/O
    d_ws = nc.dram_tensor("wsyn", [128, KC * N], MMD, kind="ExternalInput")
    d_wf = nc.dram_tensor("wfast", [128, KC * N], MMD, kind="ExternalInput")
    d_xp = nc.dram_tensor("xp", [128, TW], F32, kind="ExternalInput")
    d_ot = nc.dram_tensor("o10t", [128, 2 * KC], MMD, kind="ExternalInput")
    d_ey = nc.dram_tensor("eye32", [128, 8], F32, kind="ExternalInput")
    d_zr = nc.dram_tensor("zr", [128, 64], MMD, kind="ExternalInput")
    d_spk = nc.dram_tensor("spk", [128, TW], F32, kind="ExternalOutput")
    d_ro = nc.dram_tensor("ro", [2, nsteps * BL], F32, kind="ExternalOutput")

    # persistent SBUF
    w_s = nc.alloc_sbuf_tensor("w_s", [128, KC * N], MMD)
    w_f = nc.alloc_sbuf_tensor("w_f", [128, KC * N], MMD)
    xp_s = nc.alloc_sbuf_tensor("xp_s", [128, TW], F32)
    s_hist = nc.alloc_sbuf_tensor("s_hist", [128, nsteps + 1, KC, BL], MMD)
    spk_s = nc.alloc_sbuf_tensor("spk_s", [128, TW], F32)
    ot_s = nc.alloc_sbuf_tensor("ot_s", [128, 2 * KC], MMD)
    ey_s = nc.alloc_sbuf_tensor("ey_s", [128, 8], F32)
    ro_s = nc.alloc_sbuf_tensor("ro_s", [2, nsteps * BL], F32)

    def pair(name, shape):
        return [nc.alloc_sbuf_tensor(f"{name}{i}", shape, F32) for i in range(2)]

    gdvb = [nc.alloc_sbuf_tensor(f"gdv{i}", [128, 64], MMD)
            for i in range(2)]
    vb = pair("v", [128, 64])
    foldb = pair("fold", [128, CW])
    i10b = pair("i10", [128, 64])
    dvb = pair("dv", [128, 64])
    vnb = pair("vn", [128, 64])
    gb = pair("g", [128, 64])
    t2b = pair("t2", [128, 64])
    ggb = pair("gg", [128, 64])
    one_c = nc.alloc_sbuf_tensor("one_c", [128, 64], F32)
    two_c = nc.alloc_sbuf_tensor("two_c", [128, 64], F32)

    psA = [nc.alloc_psum_tensor(f"psA{i}", [128, CW], F32) for i in range(2)]
    psB = [nc.alloc_psum_tensor(f"psB{i}", [128, 64], F32) for i in range(2)]
    psR = nc.alloc_psum_tensor("psR", [2, 512], F32)

    AT = mybir.AluOpType

    with tile.TileContext(nc) as tc:
        # input DMAs
        nc.sync.dma_start(w_s.ap(), d_ws.ap())
        nc.sync.dma_start(w_f.ap(), d_wf.ap())
        nc.sync.dma_start(xp_s.ap(), d_xp.ap())
        nc.sync.dma_start(ot_s.ap(), d_ot.ap())
        nc.sync.dma_start(ey_s.ap(), d_ey.ap())

        # zero init (DMA for f32r-typed tensors: memset can't emit f32r)
        nc.sync.dma_start(gdvb[0].ap(), d_zr.ap())
        nc.sync.dma_start(gdvb[1].ap(), d_zr.ap())
        nc.sync.dma_start(s_hist.ap()[:, 0], d_zr.ap())
        nc.vector.memset(one_c.ap(), 1.0)
        nc.vector.memset(two_c.ap(), 2.0)
        nc.vector.memset(vb[0].ap(), 0.0)
        nc.vector.memset(vb[1].ap(), 0.0)
        nc.vector.memset(psA[0].ap(), 0.0)
        nc.vector.memset(psA[1].ap(), 0.0)

        for t in range(nsteps):
            cur, prv = t % 2, (t + 1) % 2
            pA = psA[cur].ap()
            pB = psB[cur].ap()
            xp_t = xp_s.ap()[:, t * 64:(t + 1) * 64]
            v_prv = vb[prv].ap()

            # matmuls: accumulate I/10 into pA (folded col-tiled layout)
            for k in range(KC):
                for wi, wt in enumerate((w_s, w_f)):
                    if wi == 0:
                        lhsT = s_hist.ap()[:, t, k, :]
                    else:
                        lhsT = gdvb[prv].ap()[:, k * 8:(k + 1) * 8]
                    for ct in range(NT):
                        for sub in range(max(1, CW // 512)):
                            sw = min(512, CW)
                            c0 = ct * CW + sub * sw
                            nc.tensor.matmul(
                                pA[32 * ct:32 * ct + 8, sub * sw:(sub + 1) * sw],
                                lhsT,
                                wt.ap()[:, k * N + c0: k * N + c0 + sw],
                                start=(k == 0 and wi == 0),
                                stop=(k == KC - 1 and wi == 1),
                                tile_position=(0, 32 * ct),
                                skip_group_check=True,
                            )

            # PSUM -> SBUF (folded), then 8 PE transposes -> pB [128, 64]
            fold = foldb[cur].ap()
            half = CW // 2
            nc.vector.tensor_copy(fold[:, 0:half], pA[:, 0:half])
            nc.scalar.mul(fold[:, half:CW], pA[:, half:CW], 1.0)
            for k in range(KC):
                bp = 32 * (k // PER_T)
                off = (k % PER_T) * 128
                tsrc = fold[bp:bp + 8, off:off + 128]
                nc.tensor.transpose(pB[:, k * 8:(k + 1) * 8], tsrc,
                                    ey_s.ap()[bp:bp + 8, :],
                                    tile_position=(bp, 0))

            # on-path update chain (DVE)
            i10, dv = i10b[cur].ap(), dvb[cur].ap()
            vn, g = vnb[cur].ap(), gb[cur].ap()
            nc.vector.tensor_tensor(i10, pB, xp_t, AT.add)
            nc.vector.scalar_tensor_tensor(dv, v_prv, -0.1, i10,
                                           AT.mult, AT.add)
            nc.vector.scalar_tensor_tensor(vn, v_prv, 0.9, i10,
                                           AT.mult, AT.add)
            nc.vector.tensor_scalar(g, vn, -1.0, 1.0, AT.max, AT.min)
            nc.vector.tensor_tensor(gdvb[cur].ap(), g, dv, AT.mult)
            nc.vector.scalar_tensor_tensor(
                s_hist.ap()[:, t + 1], s_hist.ap()[:, t], 0.9,
                gdvb[cur].ap(), AT.mult, AT.add)

            # off-path: spikes and v update
            # spiked <=> |vn| >= 1 <=> g*g >= 1 (g = clamp(vn, -1, 1))
            spk_t = spk_s.ap()[:, t * 64:(t + 1) * 64]
            gg = ggb[cur].ap()
            nc.gpsimd.tensor_tensor(gg, g, g, AT.mult)
            nc.vector.tensor_scalar(spk_t, gg, 1.0, None, AT.is_ge)
            nc.gpsimd.tensor_tensor(t2b[cur].ap(), two_c.ap(), spk_t,
                                    AT.subtract)
            nc.gpsimd.tensor_tensor(vb[cur].ap(), t2b[cur].ap(), vn, AT.mult)

        # readout: ro[o, t*8+b] = sum_k O10T[k].T @ S_hist[t+1]
        nh = max(1, (nsteps * BL) // 512)
        hw_ = min(512, nsteps * BL)
        for h in range(nh):
            for k in range(KC):
                lhsT = ot_s.ap()[:, k * 2:(k + 1) * 2]
                rhs = s_hist.ap()[:, 1 + h * (hw_ // 8): 1 + (h + 1) * (hw_ // 8), k, :]
                nc.tensor.matmul(psR.ap()[:, 0:hw_], lhsT, rhs,
                                 start=(k == 0), stop=(k == KC - 1),
                                 skip_group_check=True)
            nc.vector.tensor_copy(ro_s.ap()[:, h * hw_:(h + 1) * hw_],
                                  psR.ap()[:, 0:hw_])

        # outputs
        nc.sync.dma_start(d_spk.ap(), spk_s.ap())
        nc.sync.dma_start(d_ro.ap(), ro_s.ap())

    nc.compile()
    return nc


def prep_inputs(x_in, W_syn, W_fast, W_in, O, nsteps=T):
    x_in = np.asarray(x_in, dtype=np.float32)
    W_syn = np.asarray(W_syn, dtype=np.float32)
    W_fast = np.asarray(W_fast, dtype=np.float32)
    W_in = np.asarray(W_in, dtype=np.float32)
    O = np.asarray(O, dtype=np.float32)

    mask = 1.0 - np.eye(N, dtype=np.float32)
    ws = ((W_syn * mask) / 100.0).astype(np.float32)
    wf = ((W_fast * mask) / 10.0).astype(np.float32)
    # [p, k*1024+n] = W[k*128+p, n]
    ws_l = round_f32r(
        ws.reshape(KC, 128, N).transpose(1, 0, 2).reshape(128, KC * N))
    wf_l = round_f32r(
        wf.reshape(KC, 128, N).transpose(1, 0, 2).reshape(128, KC * N))

    XP = ((x_in[:nsteps].reshape(nsteps * B, 2) @ W_in)
          .reshape(nsteps, B, N) / 10.0).astype(np.float32)

    ot = (O / 10.0).astype(np.float32)
    ot_l = round_f32r(
        ot.reshape(2, KC, 128).transpose(2, 1, 0).reshape(128, 2 * KC))
    p = np.arange(128)
    eye32 = (p[:, None] % 32 == np.arange(8)[None, :]).astype(np.float32)
    zr = np.zeros((128, 64), np.float32)

    in_maps = []
    for c in range(NCORES):
        xc = XP[:, c * BL:(c + 1) * BL, :]              # [t, b, n]
        xc = xc.reshape(nsteps, BL, KC, 128)            # [t, b, k, p]
        xc = np.ascontiguousarray(
            xc.transpose(3, 0, 2, 1).reshape(128, nsteps * 64))
        in_maps.append({
            "wsyn": ws_l, "wfast": wf_l, "xp": xc,
            "o10t": ot_l, "eye32": eye32, "zr": zr,
        })
    return in_maps


def assemble(results, nsteps=T):
    spikes = np.empty((nsteps, B, N), dtype=np.float32)
    readout = np.empty((nsteps, B, 2), dtype=np.float32)
    for c in range(NCORES):
        spk = results[c]["spk"].reshape(128, nsteps, KC, BL)   # [p, t, k, b]
        spikes[:, c * BL:(c + 1) * BL, :] = (
            spk.transpose(1, 3, 2, 0).reshape(nsteps, BL, N))
        ro = results[c]["ro"].reshape(2, nsteps, BL)           # [o, t, b]
        readout[:, c * BL:(c + 1) * BL, :] = ro.transpose(1, 2, 0)
    return spikes, readout


_NC_CACHE = {}


def kernel(x_in, W_syn, W_fast, W_in, O):
    nsteps = x_in.shape[0]
    if nsteps not in _NC_CACHE:
        _NC_CACHE[nsteps] = build(nsteps)
    nc = _NC_CACHE[nsteps]
    in_maps = prep_inputs(x_in, W_syn, W_fast, W_in, O, nsteps)
    res = bass_utils.run_bass_kernel_spmd(
        nc, in_maps, core_ids=list(range(NCORES)))
    return assemble(res.results, nsteps)
